# revision 4
# baseline (speedup 1.0000x reference)
"""Trainium2 Bass kernel for nn_Encoder_48017734369665 (PointNet-style
segment-reduce encoder).

Network (B=16 clouds, N=131072 points, ragged via npts):
    h  = relu(bn1(W1 @ x))            [128, N]
    f  = bn2(W2 @ h)                  [256, N]
    g  = segment_max(f)               [B, 256]
    h3 = relu(bn3(W3 @ [g[seg]; f]))  [512, N]
    h4 = bn4(W4 @ h3)                 [1024, N]
    out = segment_max(h4)             [B, 1024]

Strategy:
  * Inference-mode batchnorms fold into the affine layers on the host.
  * W3 splits into W3g (acting on the per-cloud constant g) and W3f (acting
    on f); the W3g @ g + b3 term is a per-cloud bias vector c computed
    on-device once per cloud from tiny matmuls.
  * Each core owns 2 whole clouds (paired large-with-small so every core's
    padded point count is equal). Each cloud's columns are padded to a
    multiple of F=512 by replicating one of its real points — padding is
    then provably max-invariant and no masking of the point axis is needed.
  * Two passes over the core's tiles: pass 1 computes per-tile maxes of
    W2'h (layers 1-2); a small on-device combine turns those into per-cloud
    g (mask rows broadcast via K=1 matmuls into PSUM), then c, then a
    per-tile bias table C. Pass 2 recomputes layers 1-2 and runs layers 3-4,
    emitting per-tile maxes of W4'h3. Host combines per-tile maxes into the
    final [B, 1024] (adding b4'), so there is no cross-device communication.
  * All big matmuls run as float32r (full-rate fp32 path, free dim 512).
  * PSUM pairs (2-bank tiles) let one DVE reduce cover two matmul outputs.
"""

import numpy as np
import ml_dtypes

BF16 = ml_dtypes.bfloat16
EPS = 1e-5
B = 16
N = 131072
F = 512  # points per tile (fp32 moving-operand / PSUM-bank limit)
NCORES = 8
NEG = -1.0e30


def _fold_bn(W, b, g, be, m, v):
    """bn(W@x + b) == W' @ x + b' with W' = s*W, b' = s*(b-m)+be, s=g/sqrt(v+eps)."""
    s = g / np.sqrt(v + EPS)
    return (s[:, None] * W).astype(np.float32), (s * (b - m) + be).astype(np.float32)


def _cloud_ranges(npts):
    """Per-cloud [start, end) column ranges exactly as the reference's
    jnp.repeat(..., total_repeat_length=N) maps points to clouds: truncate
    if sum > N, extend the last cloud if sum < N."""
    npts = np.maximum(np.asarray(npts, np.int64), 0)
    ends = np.minimum(np.cumsum(npts), N)
    starts = np.concatenate([[0], ends[:-1]])
    ends = ends.copy()
    ends[-1] = N  # pad semantics: trailing points belong to the last cloud
    return [(int(s), int(e)) for s, e in zip(starts, ends)]


def _plan(npts):
    """Pair clouds 2-per-core so padded per-core tile counts are equal.

    Returns (T, slots) where slots[c] = list of (cloud_id, col_range,
    n_tiles) for core c, and sum(n_tiles) == T for every core.
    """
    ranges = _cloud_ranges(npts)
    sizes = [e - s for s, e in ranges]
    order = np.argsort(sizes)[::-1]  # big → small
    pairs = [(int(order[2 * NCORES - 1 - i]), int(order[i])) for i in range(NCORES)]
    ktiles = [max(1, -(-sizes[b] // F)) for b in range(2 * NCORES)]
    T = max(ktiles[a] + ktiles[b] for a, b in pairs)
    slots = []
    for a, b in pairs:
        ka = ktiles[a]
        kb = T - ka  # give cloud B all remaining tiles (pad extra there)
        slots.append([(a, ranges[a], ka), (b, ranges[b], kb)])
    return T, slots


def _core_inputs(x, T, core_slots, weights):
    """Build the per-core input dict (xs + masks); weights are shared."""
    xs = np.empty((3, T * F), BF16)
    mneg = np.full((1, 2 * T), NEG, BF16)
    m01 = np.zeros((1, 2 * T), BF16)
    t0 = 0
    for slot, (cid, (s, e), ktiles) in enumerate(core_slots):
        n = e - s
        cols = x[:, s:e] if n > 0 else x[:, :1]  # degenerate empty cloud
        n = max(n, 1)
        pad = ktiles * F - n
        if pad > 0:
            cols = np.concatenate([cols, np.repeat(cols[:, :1], pad, axis=1)], axis=1)
        xs[:, t0 * F : (t0 + ktiles) * F] = cols.astype(BF16)
        mneg[0, slot * T + t0 : slot * T + t0 + ktiles] = 0.0
        m01[0, slot * T + t0 : slot * T + t0 + ktiles] = 1.0
        t0 += ktiles
    weights = dict(weights)
    rowcat = np.concatenate([weights.pop("rowpre"), mneg], axis=1)
    return dict(xs=xs, rowcat=np.ascontiguousarray(rowcat), m01=m01, **weights)


def _build_nc(T, KA, KB):
    """Cloud-pipelined variant: pass 1 for late tiles rides inside pass 2 of
    early tiles, so only ~KA tiles of pass 1 (DVE-bound, ~1.2us/tile) are
    exposed.

    Per-core slot-0 tile counts ka_c vary, but the SPMD program is shared, so
    phases split at FIXED indices: KA = max_c ka_c, KB = min_c ka_c.
    Tiles [0, KB) are slot-0 on every core; [KA, T) are slot-1 on every core.
    combine-0 (mask-driven) fills Cb[:, :, 0:KA); combine-1 later overwrites
    Cb[:, :, KB:T) with cT0*m01_s0 + cT1*m01_s1 (complementary masks), which
    is correct for both slots in the ambiguous middle range.
    Two PSUM pools: phase-A pool (ph 2 banks + pf 2x2 banks) closes after
    combine-0; the B/C pool runs q1 1 + phb 1 + pfb 2 + p3 2 + p4 2 = 8.
    Pass 1 stashes h and fs-m0 for every tile (ACT slack covers it); tiles
    processed in the overlap window also stash fs-m1, so only tiles
    [0, KA) need an L2-m1 recompute in pass 2.
    """
    import concourse.mybir as mybir
    import concourse.tile as tile
    from concourse import bacc

    f32 = mybir.dt.float32
    bf16 = mybir.dt.bfloat16
    AF = mybir.ActivationFunctionType
    AX = mybir.AxisListType.X

    nc = bacc.Bacc("TRN2", target_bir_lowering=False, debug=False, num_devices=NCORES)

    xs_d = nc.dram_tensor("xs", [3, T * F], bf16, kind="ExternalInput")
    w1t_d = nc.dram_tensor("w1t", [3, 128], bf16, kind="ExternalInput")
    w2t_d = nc.dram_tensor("w2t", [128, 256], bf16, kind="ExternalInput")
    w3gt_d = nc.dram_tensor("w3gt", [128, 2, 512], bf16, kind="ExternalInput")
    w3ft_d = nc.dram_tensor("w3ft", [128, 2, 512], bf16, kind="ExternalInput")
    w4t_d = nc.dram_tensor("w4t", [128, 4, 1024], bf16, kind="ExternalInput")
    bc_d = nc.dram_tensor("bcat", [128, 3], f32, kind="ExternalInput")
    rc_d = nc.dram_tensor("rowcat", [1, 640 + 2 * T], bf16, kind="ExternalInput")
    m01_d = nc.dram_tensor("m01", [1, 2 * T], bf16, kind="ExternalInput")
    vt_d = nc.dram_tensor("vt", [128, T, 8], f32, kind="ExternalOutput")

    bounds = [0, 1, 3]
    while bounds[-1] < T:
        bounds.append(min(T, bounds[-1] + max(1, (T - 3 + 3) // 4)))

    PRE = 3

    with tile.TileContext(nc) as tc:
        with (
            tc.tile_pool(name="const", bufs=1) as cp,
            tc.tile_pool(name="work", bufs=4) as wp,
        ):
            xs_chunks = []
            for a, b_ in zip(bounds, bounds[1:]):
                xs_chunks.append(
                    (a, b_, cp.tile([3, (b_ - a) * F], bf16, name=f"xs_{a}"))
                )

            def xs_col(t):
                for a, b_, tch in xs_chunks:
                    if a <= t < b_:
                        return tch[:, (t - a) * F : (t - a + 1) * F]
                raise IndexError(t)
            hs = cp.tile([128, T, F], bf16)           # h stash, all tiles
            fsm0 = cp.tile([128, T, F], bf16)         # f[0:128] stash, all tiles
            fsm1 = cp.tile([128, max(T - KA, 1), F], bf16)  # f[128:256], tiles >= KA
            w1t = cp.tile([3, 128], bf16)
            w2t = cp.tile([128, 256], bf16)
            w3gt = cp.tile([128, 2, 512], bf16)
            w3ft = cp.tile([128, 2, 512], bf16)
            w4t = cp.tile([128, 4, 1024], bf16)
            bc = cp.tile([128, 3], f32)
            rc = cp.tile([1, 640 + 2 * T], bf16)
            m01t = cp.tile([1, 2 * T], bf16)
            b1 = bc[:, 0:1]
            b2 = bc[:, 1:3]
            b3r = rc[:, 0:512]
            ones = rc[:, 512:640]
            mneg = rc[:, 640 : 640 + 2 * T]
            m01 = m01t[:]
            Mt = cp.tile([128, T, 2], f32)
            gk = cp.tile([128, 2, 2], bf16)
            cT0 = cp.tile([1, 512], bf16)
            cT1 = cp.tile([1, 512], bf16)
            Cb = cp.tile([128, 4, T], f32)
            V = cp.tile([128, T, 8], f32)

            # first x tile on the fast HWDGE sync queue so L1(0) starts ASAP
            nc.sync.dma_start(xs_chunks[0][2][:], xs_d.ap()[:, 0:F])
            for a, b_, tch in xs_chunks[1:]:
                nc.gpsimd.dma_start(tch[:], xs_d.ap()[:, a * F : b_ * F])
            nc.scalar.dma_start(w1t[:], w1t_d.ap())
            nc.scalar.dma_start(bc[:], bc_d.ap())
            nc.sync.dma_start(w2t[:], w2t_d.ap())
            nc.sync.dma_start(rc[:], rc_d.ap())
            nc.sync.dma_start(m01t[:], m01_d.ap())
            for t_sb, t_dr in ((w3gt, w3gt_d), (w3ft, w3ft_d), (w4t, w4t_d)):
                nc.sync.dma_start(t_sb[:], t_dr.ap())

            # garbage guard: slot-0 combine reads all T columns of Mt before
            # slot 1's tiles have written theirs (mask adds -1e30 to those).
            nc.vector.memset(Mt[:], 0.0)

            warm = wp.tile([128, 1], f32, tag="gtmp")
            nc.gpsimd.memset(warm[:], 0.0)
            nc.scalar.activation(warm[:], warm[:], AF.Relu, bias=warm[:])
            nc.scalar.activation(warm[:], warm[:], AF.Identity, bias=warm[:])

            def emit_l1(pool, t, htag, hbufs):
                ph = pool.tile([128, F], f32, tag=htag, bufs=hbufs,
                               name=f"p1h_{t}")
                nc.tensor.matmul(ph[:], w1t[:], xs_col(t), start=True, stop=True)
                nc.scalar.activation(hs[:, t, :], ph[:], AF.Relu, bias=b1)

            def make_pass1(pool, htag, hbufs, ftag, fbufs):
                def pass1_l2r(t, stash_m1=False):
                    if ftag is not None:
                        pf = pool.tile([128, 2, F], f32, tag=ftag, bufs=fbufs,
                                       name=f"p1f_{t}")
                        nc.tensor.matmul(
                            pf[:, 0, :], w2t[:, 0:128],
                            hs[:, t, :], start=True, stop=True,
                        )
                        nc.tensor.matmul(
                            pf[:, 1, :], w2t[:, 128:256],
                            hs[:, t, :], start=True, stop=True,
                        )
                        nc.vector.reduce_max(Mt[:, t, :], pf[:], axis=AX)
                        pm = (pf[:, 0, :], pf[:, 1, :])
                    else:
                        # single-bank psums from the shared ring, 2 reduces
                        pm = []
                        for m in range(2):
                            p = pool.tile([128, F], f32, tag=htag, bufs=hbufs,
                                          name=f"p1f_{t}_{m}")
                            nc.tensor.matmul(
                                p[:], w2t[:, m * 128 : (m + 1) * 128],
                                hs[:, t, :], start=True, stop=True,
                            )
                            nc.vector.reduce_max(
                                Mt[:, t, m : m + 1], p[:], axis=AX
                            )
                            pm.append(p[:])
                    nc.scalar.activation(
                        fsm0[:, t, :], pm[0], AF.Identity, bias=b2[:, 0:1]
                    )
                    if stash_m1:
                        nc.scalar.activation(
                            fsm1[:, t - KA, :], pm[1], AF.Identity,
                            bias=b2[:, 1:2],
                        )
                return pass1_l2r

            def combine_g(pool, qtag, qb, sl, cTs):
                """Per-cloud g for slot sl over ALL T tile columns (the
                slot mask kills foreign/unwritten tiles), then c."""
                pmask = pool.tile([128, 2 * T], f32, tag=qtag, bufs=qb)
                nc.tensor.matmul(pmask[:], ones, mneg, start=True, stop=True)
                for m in range(2):
                    cmb = wp.tile([128, T], f32, tag="cmb")
                    nc.vector.tensor_add(
                        cmb[:], Mt[:, :, m], pmask[:, sl * T : (sl + 1) * T]
                    )
                    gtmp = wp.tile([128, 1], f32, tag="gtmp")
                    nc.vector.reduce_max(gtmp[:], cmb[:], axis=AX)
                    nc.vector.tensor_add(
                        gk[:, m, sl : sl + 1], gtmp[:], b2[:, m : m + 1]
                    )
                pcT = pool.tile([1, 512], f32, tag=qtag, bufs=qb)
                nc.tensor.matmul(
                    pcT[:], gk[:, 0, sl : sl + 1], w3gt[:, 0, :],
                    start=True, stop=False,
                )
                nc.tensor.matmul(
                    pcT[:], gk[:, 1, sl : sl + 1], w3gt[:, 1, :],
                    start=False, stop=False,
                )
                nc.tensor.matmul(pcT[:], ones[:, 0:1], b3r, start=False, stop=True)
                nc.vector.tensor_copy(cTs[:], pcT[:])

            def fill_cb(pool, qtag, qb, t0, t1, both):
                """Cb[:, :, t0:t1) from cT0 (and cT1 when both=True)."""
                for mo in range(4):
                    pC = pool.tile([128, T], f32, tag=qtag, bufs=qb)
                    nc.tensor.matmul(
                        pC[:, 0 : t1 - t0], cT0[:, mo * 128 : (mo + 1) * 128],
                        m01[:, t0:t1], start=True, stop=not both,
                    )
                    if both:
                        nc.tensor.matmul(
                            pC[:, 0 : t1 - t0],
                            cT1[:, mo * 128 : (mo + 1) * 128],
                            m01[:, T + t0 : T + t1], start=False, stop=True,
                        )
                    nc.vector.tensor_copy(Cb[:, mo, t0:t1], pC[:, 0 : t1 - t0])

            fss = {}

            def l2(pool, t):
                """Recompute f[128:256] for tile t (m0 comes from fsm0)."""
                fs = wp.tile([128, F], bf16, tag="f", bufs=PRE + 2,
                             name=f"fs_{t}")
                p2 = pool.tile([128, F], f32, tag="pb", bufs=2, name=f"p2_{t}")
                nc.tensor.matmul(
                    p2[:], w2t[:, 128:256], hs[:, t, :], start=True, stop=True,
                )
                nc.scalar.activation(fs[:], p2[:], AF.Identity, bias=b2[:, 1:2])
                fss[t] = fs

            def pass2_tile(pool, t, pre_next):
                if t < KA:
                    fs1 = fss.pop(t)[:]
                else:
                    fs1 = fsm1[:, t - KA, :]
                fks = (fsm0[:, t, :], fs1)
                h3 = wp.tile([128, 4, F], bf16, tag="h3", bufs=2, name=f"h3_{t}")
                for mo in range(4):
                    p3 = pool.tile([128, F], f32, tag="p3", bufs=3,
                                   name=f"p3_{t}_{mo}")
                    for k in range(2):
                        nc.tensor.matmul(
                            p3[:], w3ft[:, k, mo * 128 : (mo + 1) * 128],
                            fks[k], start=(k == 0), stop=(k == 1),
                        )
                    nc.scalar.activation(
                        h3[:, mo, :], p3[:], AF.Relu, bias=Cb[:, mo, t : t + 1]
                    )
                if pre_next is not None:
                    l2(pool, pre_next)
                for mo in range(8):
                    p4 = pool.tile([128, F], f32, tag="p4", bufs=3,
                                   name=f"p4_{t}_{mo}")
                    for k in range(4):
                        nc.tensor.matmul(
                            p4[:], w4t[:, k, mo * 128 : (mo + 1) * 128],
                            h3[:, k, :], start=(k == 0), stop=(k == 3),
                        )
                    nc.vector.reduce_max(V[:, t, mo : mo + 1], p4[:], axis=AX)
                if t % 8 == 7 or t == T - 1:
                    a = (t // 8) * 8
                    nc.sync.dma_start(
                        vt_d.ap()[:, a : t + 1, :], V[:, a : t + 1, :]
                    )

            # ---- phase A: pass 1 over tiles [0, KA), L1 one tile ahead ---
            with tc.tile_pool(name="psumA", bufs=1, space="PSUM") as ppA:

                def l2A(t):
                    """l2 prefetch inside phase A (spare psumA bank), so the
                    combine->phase-B transition starts with fs already hot."""
                    fs = wp.tile([128, F], bf16, tag="f", bufs=PRE + 2,
                                 name=f"fs_{t}")
                    p2 = ppA.tile([128, F], f32, tag="q2", bufs=1,
                                  name=f"p2a_{t}")
                    nc.tensor.matmul(
                        p2[:], w2t[:, 128:256], hs[:, t, :],
                        start=True, stop=True,
                    )
                    nc.scalar.activation(
                        fs[:], p2[:], AF.Identity, bias=b2[:, 1:2]
                    )
                    fss[t] = fs

                p1A = make_pass1(ppA, "ph", 3, "pf", 2)
                emit_l1(ppA, 0, "ph", 3)
                for t in range(KA):
                    if t + 1 < KA:
                        emit_l1(ppA, t + 1, "ph", 3)
                    p1A(t)
                    # one l2 prefetch every other tile near the end of phase A
                    u = (t - (KA - 2 * PRE)) // 2
                    if t >= KA - 2 * PRE and (t - KA) % 2 == 0 and 0 <= u < PRE:
                        l2A(u)
                combine_g(ppA, "ph", 3, 0, cT0)

            # ---- phases B/C ----------------------------------------------
            with tc.tile_pool(name="psumB", bufs=1, space="PSUM") as ppB:
                p1B_l2r = make_pass1(ppB, "pb", 2, None, 2)

                def p1B(t, stash_m1=False):
                    emit_l1(ppB, t, "pb", 2)
                    p1B_l2r(t, stash_m1)
                for t in range(min(PRE, KB)):
                    if t not in fss:
                        l2(ppB, t)
                fill_cb(ppB, "pb", 2, 0, KA, both=False)

                # phase B: pass 2 over [0, KB), pass 1 over [KA, T)
                bptr = KA
                bdone_emitted = False
                for t in range(KB):
                    nxt = t + PRE
                    pass2_tile(ppB, t, nxt if nxt < KA else None)
                    while bptr < T and bptr - KA < 2 * (t + 1):
                        p1B(bptr, stash_m1=True)
                        bptr += 1
                    if bptr >= T and not bdone_emitted:
                        combine_g(ppB, "pb", 2, 1, cT1)
                        fill_cb(ppB, "pb", 2, KB, T, both=True)
                        bdone_emitted = True
                if not bdone_emitted:
                    while bptr < T:
                        p1B(bptr, stash_m1=True)
                        bptr += 1
                    combine_g(ppB, "pb", 2, 1, cT1)
                    fill_cb(ppB, "pb", 2, KB, T, both=True)

                # phase C: pass 2 over [KB, T)
                for t in range(KB, T):
                    if t < KA and t not in fss:
                        l2(ppB, t)
                    nxt = t + PRE
                    pass2_tile(ppB, t, nxt if nxt < KA and nxt not in fss else None)

    nc.compile()
    return nc


def _prep(x, npts, W1, b1, g1, be1, m1, v1, W2, b2, g2, be2, m2, v2,
          W3, b3, g3, be3, m3, v3, W4, b4, g4, be4, m4, v4):
    """Host-side preprocessing shared by kernel() and the test harness."""
    W1f, b1f = _fold_bn(W1, b1, g1, be1, m1, v1)
    W2f, b2f = _fold_bn(W2, b2, g2, be2, m2, v2)
    W3f_, b3f = _fold_bn(W3, b3, g3, be3, m3, v3)
    W4f, b4f = _fold_bn(W4, b4, g4, be4, m4, v4)

    weights = dict(
        w1t=np.ascontiguousarray(W1f.T).astype(BF16),
        w2t=np.ascontiguousarray(W2f.T).astype(BF16),
        w3gt=np.ascontiguousarray(W3f_[:, :256].T.reshape(2, 128, 512).transpose(1, 0, 2)).astype(BF16),
        w3ft=np.ascontiguousarray(W3f_[:, 256:].T.reshape(2, 128, 512).transpose(1, 0, 2)).astype(BF16),
        w4t=np.ascontiguousarray(W4f.T.reshape(4, 128, 1024).transpose(1, 0, 2)).astype(BF16),
        bcat=np.ascontiguousarray(
            np.concatenate([b1f[:, None], b2f.reshape(2, 128).T], axis=1)
        ),
        rowpre=np.concatenate(
            [b3f[None, :], np.ones((1, 128), np.float32)], axis=1
        ).astype(BF16),
    )

    T, slots = _plan(npts)
    kas = [core_slots[0][2] for core_slots in slots]
    KA, KB = max(kas), min(kas)
    x = np.asarray(x, np.float32)
    in_maps = [_core_inputs(x, T, slots, weights) for slots in slots]
    return (T, KA, KB), slots, in_maps, b4f


def _gather(results, T, slots, b4f):
    """Combine per-core per-tile maxes into the [B, 1024] output."""
    if isinstance(T, tuple):
        T = T[0]
    out = np.empty((B, 1024), np.float32)
    for c, core_slots in enumerate(slots):
        vt = results[c]["vt"]  # [128, T, 8]; channel = mo*128 + partition
        chan = vt.transpose(2, 0, 1).reshape(1024, T)
        t0 = 0
        for cid, _rng, ktiles in core_slots:
            out[cid] = chan[:, t0 : t0 + ktiles].max(axis=1) + b4f
            t0 += ktiles
    return out


def kernel(**inputs):
    from concourse.bass_utils import run_bass_kernel_spmd

    # force host numpy: jax arrays would route host math through the (axon)
    # device backend
    inputs = {k: np.asarray(v) for k, v in inputs.items()}
    T, slots, in_maps, b4f = _prep(**inputs)
    nc = _build_nc(*T)
    res = run_bass_kernel_spmd(nc, in_maps, core_ids=list(range(NCORES)))
    return _gather(res.results, T, slots, b4f)

